# revision 17
# baseline (speedup 1.0000x reference)
"""Distributed single-head attention block for trn2 (8 NeuronCores).

reference:
    q = x @ Wq.T + bq ; k = x @ Wk.T + bk ; v = x @ Wv.T + bv
    out = x + softmax(q @ k.T / sqrt(D)) @ v       x: [4, 2048, 1024]

Sharding: 8 cores = 4 batches x 2 query-halves. Core c owns batch c//2 and
query rows [h*1024, (h+1)*1024) with h = c%2. Each core recomputes K/V for
its whole batch (duplicated across the pair; no collectives needed).

Device-side layouts (host pre-transposes + bf16-casts so the contraction
dim always lands on SBUF partitions):
    xT  [D, S]   bf16   x[b].T            -> K/V projections
    xqT [D, SQ]  bf16   x[b, half].T      -> Q projection
    xq  [SQ, D]  f32    x[b, half]        -> residual add
    w*T [D, D]   bf16   W.T
Projections emit qT/kT [e, s] (scores contraction over e) and v [s, e]
(attn contraction over keys). Softmax rows live on partitions: exp on
ScalarE with accum_out giving row sums for free; no max subtraction
(scores are O(10) for this model so exp cannot overflow in f32).
P is transposed 128x128 on TensorE (identity matmul) for the attn matmul.
"""

import numpy as np

B, S, D = 4, 2048, 1024
SQ = S // 2  # queries per core
NCORES = 8
DC = D // 128  # contraction chunks
EC = D // 128  # embed chunks
SC = S // 128  # key chunks
QT = SQ // 128  # query tiles per core

_cache = {}


def _build():
    import concourse.bass as bass
    import concourse.tile as tile
    from concourse import bacc, mybir
    from concourse.masks import make_identity

    f32 = mybir.dt.float32
    bf16 = mybir.dt.bfloat16
    Alu = mybir.AluOpType
    Act = mybir.ActivationFunctionType

    nc = bacc.Bacc(None, target_bir_lowering=False, debug=False)

    xT_d = nc.declare_dram_parameter("xT", [D, S], bf16, isOutput=False)
    xqT_d = nc.declare_dram_parameter("xqT", [D, SQ], bf16, isOutput=False)
    xq_d = nc.declare_dram_parameter("xq", [SQ, D], f32, isOutput=False)
    wq_d = nc.declare_dram_parameter("wqT", [D, D], bf16, isOutput=False)
    wk_d = nc.declare_dram_parameter("wkT", [D, D], bf16, isOutput=False)
    wv_d = nc.declare_dram_parameter("wvT", [D, D], bf16, isOutput=False)
    bq_d = nc.declare_dram_parameter("bq", [D], f32, isOutput=False)
    bk_d = nc.declare_dram_parameter("bk", [D], f32, isOutput=False)
    bv_d = nc.declare_dram_parameter("bv", [D], f32, isOutput=False)
    out_d = nc.declare_dram_parameter("out", [SQ, D], f32, isOutput=True)

    # V is projected only for this core's own key-half; the pair exchanges
    # halves via a pairwise AllGather. The AG slot order (rank0, rank1)
    # equals natural batch order, identical on both cores, so the readback
    # is uniform across the SPMD graph.
    vx_in = nc.dram_tensor("vx_in", [SC // 2, 128, D], bf16)
    vx_out = nc.dram_tensor("vx_out", [2, SC // 2, 128, D], bf16)

    with tile.TileContext(nc) as tc:
        with tc.tile_pool(name="pers", bufs=1) as pers:
            qT_sb = pers.tile([128, EC, SQ], bf16, tag="qT")
            kT_sb = pers.tile([128, EC, S], bf16, tag="kT")
            v_sb = pers.tile([128, SC, D], bf16, tag="v")
            ident = pers.tile([128, 128], bf16, tag="ident")
            make_identity(nc, ident)
            # biases: bq/bk consumed per-partition in [e, s] layouts;
            # bv broadcast across partitions for the [s, e] layout.
            bq_sb = pers.tile([128, EC], f32, tag="bq")
            bk_sb = pers.tile([128, EC], f32, tag="bk")
            bv_sb = pers.tile([128, D], f32, tag="bv")

            # PE warmup: dense dummy matmuls while the first input DMAs land,
            # so the HAM clock gate is already at 2.4GHz when real work starts.
            warm_sb = pers.tile([128, 512], bf16, tag="warm")
            warm_dump = pers.tile([128, 512], f32, tag="warm_dump")
            nc.vector.memset(warm_sb, 0.0)
            with tc.tile_pool(name="warm_ps", bufs=1, space="PSUM") as warm_ps:
                wps = warm_ps.tile([128, 512], f32, tag="wps")
                NWARM = 16
                for i in range(NWARM):
                    nc.tensor.matmul(
                        wps,
                        lhsT=warm_sb[:, 0:128],
                        rhs=warm_sb,
                        start=(i == 0),
                        stop=(i == NWARM - 1),
                    )
                nc.vector.tensor_copy(out=warm_dump, in_=wps)

            bq_ap = bq_d.ap()
            bk_ap = bk_d.ap()
            bv_ap = bv_d.ap()
            nc.scalar.dma_start(
                out=bq_sb,
                in_=bass.AP(tensor=bq_ap.tensor, offset=0, ap=[[1, 128], [128, EC]]),
            )
            nc.scalar.dma_start(
                out=bk_sb,
                in_=bass.AP(tensor=bk_ap.tensor, offset=0, ap=[[1, 128], [128, EC]]),
            )
            nc.scalar.dma_start(
                out=bv_sb,
                in_=bass.AP(tensor=bv_ap.tensor, offset=0, ap=[[0, 128], [1, D]]),
            )

            with (
                tc.tile_pool(name="ld", bufs=1) as ld,
                tc.tile_pool(name="proj_ps", bufs=6, space="PSUM") as proj_ps,
            ):
                xT_sb = ld.tile([128, DC, S], bf16, tag="xT")
                xqT_sb = ld.tile([128, DC, SQ], bf16, tag="xqT")
                wq_sb = ld.tile([128, DC, D], bf16, tag="wq")
                wk_sb = ld.tile([128, DC, D], bf16, tag="wk")
                wv_sb = ld.tile([128, DC, D], bf16, tag="wv")
                # DMA priority order: v-own projection inputs first (they gate
                # the first matmuls AND the pairwise exchange), then k inputs,
                # then q weights.
                for dc in range(DC):
                    r = slice(dc * 128, (dc + 1) * 128)
                    nc.sync.dma_start(out=xqT_sb[:, dc, :], in_=xqT_d[r, :])
                    nc.sync.dma_start(out=wv_sb[:, dc, :], in_=wv_d[r, :])
                for dc in range(DC):
                    r = slice(dc * 128, (dc + 1) * 128)
                    nc.sync.dma_start(out=xT_sb[:, dc, :], in_=xT_d[r, :])
                    nc.sync.dma_start(out=wk_sb[:, dc, :], in_=wk_d[r, :])
                for dc in range(DC):
                    r = slice(dc * 128, (dc + 1) * 128)
                    nc.sync.dma_start(out=wq_sb[:, dc, :], in_=wq_d[r, :])

                # v-own [sk_own, e] for this core's key half, staged to DRAM
                # for the pairwise exchange
                with tc.tile_pool(name="vstage", bufs=8) as vstage:
                    for sc in range(SC // 2):
                        vt = vstage.tile([128, D], bf16, tag="vt")
                        for j in range(D // 512):
                            ps = proj_ps.tile([128, 512], f32, tag="ps")
                            for dc in range(DC):
                                nc.tensor.matmul(
                                    ps,
                                    lhsT=xqT_sb[:, dc, sc * 128 : (sc + 1) * 128],
                                    rhs=wv_sb[:, dc, j * 512 : (j + 1) * 512],
                                    start=(dc == 0),
                                    stop=(dc == DC - 1),
                                )
                            nc.vector.tensor_add(
                                vt[:, j * 512 : (j + 1) * 512],
                                ps,
                                bv_sb[:, j * 512 : (j + 1) * 512],
                            )
                        nc.scalar.dma_start(out=vx_in[sc], in_=vt)
                    nc.gpsimd.collective_compute(
                        "AllGather",
                        mybir.AluOpType.bypass,
                        replica_groups=[[0, 1], [2, 3], [4, 5], [6, 7]],
                        ins=[vx_in.ap().opt()],
                        outs=[vx_out.ap().opt()],
                    )
                    for r2 in range(2):
                        for sc in range(SC // 2):
                            nc.scalar.dma_start(
                                out=v_sb[:, r2 * (SC // 2) + sc, :],
                                in_=vx_out[r2, sc],
                            )

                # kT[e, sk] over the full batch
                for ec in range(EC):
                    for j in range(S // 512):
                        ps = proj_ps.tile([128, 512], f32, tag="ps")
                        for dc in range(DC):
                            nc.tensor.matmul(
                                ps,
                                lhsT=wk_sb[:, dc, ec * 128 : (ec + 1) * 128],
                                rhs=xT_sb[:, dc, j * 512 : (j + 1) * 512],
                                start=(dc == 0),
                                stop=(dc == DC - 1),
                            )
                        nc.vector.tensor_scalar_add(
                            out=kT_sb[:, ec, j * 512 : (j + 1) * 512],
                            in0=ps,
                            scalar1=bk_sb[:, ec : ec + 1],
                        )
                # qT[e, sq] = sum_d wqT[d, e] * xqT[d, sq]  (+bq per-partition)
                for j in range(SQ // 512):
                    for ec in range(EC):
                        ps = proj_ps.tile([128, 512], f32, tag="ps")
                        for dc in range(DC):
                            nc.tensor.matmul(
                                ps,
                                lhsT=wq_sb[:, dc, ec * 128 : (ec + 1) * 128],
                                rhs=xqT_sb[:, dc, j * 512 : (j + 1) * 512],
                                start=(dc == 0),
                                stop=(dc == DC - 1),
                            )
                        nc.vector.tensor_scalar_add(
                            out=qT_sb[:, ec, j * 512 : (j + 1) * 512],
                            in0=ps,
                            scalar1=bq_sb[:, ec : ec + 1],
                        )

            with (
                tc.tile_pool(name="att", bufs=2) as att,
                tc.tile_pool(name="small", bufs=2) as small,
                tc.tile_pool(name="score_ps", bufs=3, space="PSUM") as score_ps,
                tc.tile_pool(name="tr_ps", bufs=2, space="PSUM") as tr_ps,
                tc.tile_pool(name="attn_ps", bufs=3, space="PSUM") as attn_ps,
            ):
                inv_sqrt_d = float(1.0 / np.sqrt(D))
                for qt in range(QT):
                    qsl = slice(qt * 128, (qt + 1) * 128)
                    P_sb = att.tile([128, S], bf16, tag="P")
                    den4 = small.tile([128, S // 512], f32, tag="den4")
                    for kc in range(S // 512):
                        ps = score_ps.tile([128, 512], f32, tag="score")
                        for ec in range(EC):
                            nc.tensor.matmul(
                                ps,
                                lhsT=qT_sb[:, ec, qsl],
                                rhs=kT_sb[:, ec, kc * 512 : (kc + 1) * 512],
                                start=(ec == 0),
                                stop=(ec == EC - 1),
                            )
                        nc.scalar.activation(
                            out=P_sb[:, kc * 512 : (kc + 1) * 512],
                            in_=ps,
                            func=Act.Exp,
                            scale=inv_sqrt_d,
                            accum_out=den4[:, kc : kc + 1],
                        )
                    recip = small.tile([128, 1], f32, tag="recip")
                    den = small.tile([128, 1], f32, tag="den")
                    nc.vector.reduce_sum(out=den, in_=den4, axis=mybir.AxisListType.X)
                    nc.vector.reciprocal(recip, den)

                    PT_sb = att.tile([128, SC, 128], bf16, tag="PT")
                    for j in range(SC):
                        tp = tr_ps.tile([128, 128], bf16, tag="tr")
                        nc.tensor.transpose(tp, P_sb[:, j * 128 : (j + 1) * 128], ident)
                        nc.vector.tensor_copy(out=PT_sb[:, j, :], in_=tp)

                    xq_sb = att.tile([128, D], f32, tag="xq")
                    nc.sync.dma_start(out=xq_sb, in_=xq_d[qsl, :])
                    ot = att.tile([128, D], f32, tag="ot")
                    for j2 in range(D // 512):
                        pa = attn_ps.tile([128, 512], f32, tag="attn")
                        for j in range(SC):
                            nc.tensor.matmul(
                                pa,
                                lhsT=PT_sb[:, j, :],
                                rhs=v_sb[:, j, j2 * 512 : (j2 + 1) * 512],
                                start=(j == 0),
                                stop=(j == SC - 1),
                            )
                        # out = attn * (1/den) + residual
                        nc.vector.scalar_tensor_tensor(
                            out=ot[:, j2 * 512 : (j2 + 1) * 512],
                            in0=pa,
                            scalar=recip,
                            in1=xq_sb[:, j2 * 512 : (j2 + 1) * 512],
                            op0=Alu.mult,
                            op1=Alu.add,
                        )
                    nc.sync.dma_start(out=out_d[qsl, :], in_=ot)

    nc.compile()
    return nc


def _get_nc():
    if "nc" not in _cache:
        _cache["nc"] = _build()
    return _cache["nc"]


def kernel(embedded, Wq, bq, Wk, bk, Wv, bv):
    import ml_dtypes

    from concourse.bass_utils import run_bass_kernel_spmd

    bf16 = ml_dtypes.bfloat16
    x = np.ascontiguousarray(np.asarray(embedded, dtype=np.float32))
    Wq = np.asarray(Wq, dtype=np.float32)
    Wk = np.asarray(Wk, dtype=np.float32)
    Wv = np.asarray(Wv, dtype=np.float32)
    bq = np.ascontiguousarray(np.asarray(bq, dtype=np.float32))
    bk = np.ascontiguousarray(np.asarray(bk, dtype=np.float32))
    bv = np.ascontiguousarray(np.asarray(bv, dtype=np.float32))

    wqT = np.ascontiguousarray(Wq.T).astype(bf16)
    wkT = np.ascontiguousarray(Wk.T).astype(bf16)
    wvT = np.ascontiguousarray(Wv.T).astype(bf16)
    xT = [np.ascontiguousarray(x[b].T).astype(bf16) for b in range(B)]

    in_maps = []
    for c in range(NCORES):
        b, h = c // 2, c % 2
        qs = slice(h * SQ, (h + 1) * SQ)
        in_maps.append(
            {
                "xT": xT[b],
                "xqT": np.ascontiguousarray(xT[b][:, qs]),
                "xq": np.ascontiguousarray(x[b, qs, :]),
                "wqT": wqT,
                "wkT": wkT,
                "wvT": wvT,
                "bq": bq,
                "bk": bk,
                "bv": bv,
            }
        )

    nc = _get_nc()
    res = run_bass_kernel_spmd(nc, in_maps, core_ids=list(range(NCORES)))
    out = np.empty((B, S, D), dtype=np.float32)
    for c in range(NCORES):
        b, h = c // 2, c % 2
        out[b, h * SQ : (h + 1) * SQ, :] = res.results[c]["out"]
    return out


# revision 18
# speedup vs baseline: 1.0147x; 1.0147x over previous
"""Distributed single-head attention block for trn2 (8 NeuronCores).

reference:
    q = x @ Wq.T + bq ; k = x @ Wk.T + bk ; v = x @ Wv.T + bv
    out = x + softmax(q @ k.T / sqrt(D)) @ v       x: [4, 2048, 1024]

Sharding: 8 cores = 4 batches x 2 query-halves. Core c owns batch c//2 and
query rows [h*1024, (h+1)*1024) with h = c%2. Each core recomputes K/V for
its whole batch (duplicated across the pair; no collectives needed).

Device-side layouts (host pre-transposes + bf16-casts so the contraction
dim always lands on SBUF partitions):
    xT  [D, S]   bf16   x[b].T            -> K/V projections
    xqT [D, SQ]  bf16   x[b, half].T      -> Q projection
    xq  [SQ, D]  f32    x[b, half]        -> residual add
    w*T [D, D]   bf16   W.T
Projections emit qT/kT [e, s] (scores contraction over e) and v [s, e]
(attn contraction over keys). Softmax rows live on partitions: exp on
ScalarE with accum_out giving row sums for free; no max subtraction
(scores are O(10) for this model so exp cannot overflow in f32).
P is transposed 128x128 on TensorE (identity matmul) for the attn matmul.
"""

import numpy as np

B, S, D = 4, 2048, 1024
SQ = S // 2  # queries per core
NCORES = 8
DC = D // 128  # contraction chunks
EC = D // 128  # embed chunks
SC = S // 128  # key chunks
QT = SQ // 128  # query tiles per core

_cache = {}


def _build():
    import concourse.bass as bass
    import concourse.tile as tile
    from concourse import bacc, mybir
    from concourse.masks import make_identity

    f32 = mybir.dt.float32
    bf16 = mybir.dt.bfloat16
    Alu = mybir.AluOpType
    Act = mybir.ActivationFunctionType

    nc = bacc.Bacc(None, target_bir_lowering=False, debug=False)

    xT_d = nc.declare_dram_parameter("xT", [D, S], bf16, isOutput=False)
    xqT_d = nc.declare_dram_parameter("xqT", [D, SQ], bf16, isOutput=False)
    xq_d = nc.declare_dram_parameter("xq", [SQ, D], f32, isOutput=False)
    wq_d = nc.declare_dram_parameter("wqT", [D, D], bf16, isOutput=False)
    wk_d = nc.declare_dram_parameter("wkT", [D, D], bf16, isOutput=False)
    wv_d = nc.declare_dram_parameter("wvT", [D, D], bf16, isOutput=False)
    bq_d = nc.declare_dram_parameter("bq", [D], f32, isOutput=False)
    bk_d = nc.declare_dram_parameter("bk", [D], f32, isOutput=False)
    bv_d = nc.declare_dram_parameter("bv", [D], f32, isOutput=False)
    out_d = nc.declare_dram_parameter("out", [SQ, D], f32, isOutput=True)

    # V is projected only for this core's own key-half; the pair exchanges
    # halves via a pairwise AllGather. The AG slot order (rank0, rank1)
    # equals natural batch order, identical on both cores, so the readback
    # is uniform across the SPMD graph.
    vx_in = nc.dram_tensor("vx_in", [SC // 2, 128, D], bf16)
    vx_out = nc.dram_tensor("vx_out", [2, SC // 2, 128, D], bf16)

    with tile.TileContext(nc) as tc:
        with tc.tile_pool(name="pers", bufs=1) as pers:
            qT_sb = pers.tile([128, EC, SQ], bf16, tag="qT")
            kT_sb = pers.tile([128, EC, S], bf16, tag="kT")
            v_sb = pers.tile([128, SC, D], bf16, tag="v")
            ident = pers.tile([128, 128], bf16, tag="ident")
            make_identity(nc, ident)
            # biases: bq/bk consumed per-partition in [e, s] layouts;
            # bv broadcast across partitions for the [s, e] layout.
            bq_sb = pers.tile([128, EC], f32, tag="bq")
            bk_sb = pers.tile([128, EC], f32, tag="bk")
            bv_sb = pers.tile([128, D], f32, tag="bv")

            # PE warmup: dense dummy matmuls while the first input DMAs land,
            # so the HAM clock gate is already at 2.4GHz when real work starts.
            warm_sb = pers.tile([128, 512], bf16, tag="warm")
            warm_dump = pers.tile([128, 512], f32, tag="warm_dump")
            nc.vector.memset(warm_sb, 0.0)
            with tc.tile_pool(name="warm_ps", bufs=1, space="PSUM") as warm_ps:
                wps = warm_ps.tile([128, 512], f32, tag="wps")
                NWARM = 14
                for i in range(NWARM):
                    nc.tensor.matmul(
                        wps,
                        lhsT=warm_sb[:, 0:128],
                        rhs=warm_sb,
                        start=(i == 0),
                        stop=(i == NWARM - 1),
                    )
                nc.vector.tensor_copy(out=warm_dump, in_=wps)

            bq_ap = bq_d.ap()
            bk_ap = bk_d.ap()
            bv_ap = bv_d.ap()
            nc.scalar.dma_start(
                out=bq_sb,
                in_=bass.AP(tensor=bq_ap.tensor, offset=0, ap=[[1, 128], [128, EC]]),
            )
            nc.scalar.dma_start(
                out=bk_sb,
                in_=bass.AP(tensor=bk_ap.tensor, offset=0, ap=[[1, 128], [128, EC]]),
            )
            nc.scalar.dma_start(
                out=bv_sb,
                in_=bass.AP(tensor=bv_ap.tensor, offset=0, ap=[[0, 128], [1, D]]),
            )

            with (
                tc.tile_pool(name="ld", bufs=1) as ld,
                tc.tile_pool(name="proj_ps", bufs=4, space="PSUM") as proj_ps,
            ):
                xT_sb = ld.tile([128, DC, S], bf16, tag="xT")
                xqT_sb = ld.tile([128, DC, SQ], bf16, tag="xqT")
                wq_sb = ld.tile([128, DC, D], bf16, tag="wq")
                wk_sb = ld.tile([128, DC, D], bf16, tag="wk")
                wv_sb = ld.tile([128, DC, D], bf16, tag="wv")
                # DMA priority order: v-own projection inputs first (they gate
                # the first matmuls AND the pairwise exchange), then k inputs,
                # then q weights.
                for dc in range(DC):
                    r = slice(dc * 128, (dc + 1) * 128)
                    nc.sync.dma_start(out=xqT_sb[:, dc, :], in_=xqT_d[r, :])
                    nc.sync.dma_start(out=wv_sb[:, dc, :], in_=wv_d[r, :])
                for dc in range(DC):
                    r = slice(dc * 128, (dc + 1) * 128)
                    nc.sync.dma_start(out=xT_sb[:, dc, :], in_=xT_d[r, :])
                    nc.sync.dma_start(out=wk_sb[:, dc, :], in_=wk_d[r, :])
                for dc in range(DC):
                    r = slice(dc * 128, (dc + 1) * 128)
                    nc.sync.dma_start(out=wq_sb[:, dc, :], in_=wq_d[r, :])

                # v-own [sk_own, e] for this core's key half, staged to DRAM
                # for the pairwise exchange
                with tc.tile_pool(name="vstage", bufs=8) as vstage:
                    for sc in range(SC // 2):
                        vt = vstage.tile([128, D], bf16, tag="vt")
                        for j in range(D // 512):
                            ps = proj_ps.tile([128, 512], f32, tag="ps")
                            for dc in range(DC):
                                nc.tensor.matmul(
                                    ps,
                                    lhsT=xqT_sb[:, dc, sc * 128 : (sc + 1) * 128],
                                    rhs=wv_sb[:, dc, j * 512 : (j + 1) * 512],
                                    start=(dc == 0),
                                    stop=(dc == DC - 1),
                                )
                            nc.vector.tensor_add(
                                vt[:, j * 512 : (j + 1) * 512],
                                ps,
                                bv_sb[:, j * 512 : (j + 1) * 512],
                            )
                        nc.scalar.dma_start(out=vx_in[sc], in_=vt)
                    nc.gpsimd.collective_compute(
                        "AllGather",
                        mybir.AluOpType.bypass,
                        replica_groups=[[0, 1], [2, 3], [4, 5], [6, 7]],
                        ins=[vx_in.ap().opt()],
                        outs=[vx_out.ap().opt()],
                    )
                    for r2 in range(2):
                        for sc in range(SC // 2):
                            nc.scalar.dma_start(
                                out=v_sb[:, r2 * (SC // 2) + sc, :],
                                in_=vx_out[r2, sc],
                            )

                # kT[e, sk] over the full batch
                for ec in range(EC):
                    for j in range(S // 512):
                        ps = proj_ps.tile([128, 512], f32, tag="ps")
                        for dc in range(DC):
                            nc.tensor.matmul(
                                ps,
                                lhsT=wk_sb[:, dc, ec * 128 : (ec + 1) * 128],
                                rhs=xT_sb[:, dc, j * 512 : (j + 1) * 512],
                                start=(dc == 0),
                                stop=(dc == DC - 1),
                            )
                        nc.vector.tensor_scalar_add(
                            out=kT_sb[:, ec, j * 512 : (j + 1) * 512],
                            in0=ps,
                            scalar1=bk_sb[:, ec : ec + 1],
                        )
                # qT[e, sq] = sum_d wqT[d, e] * xqT[d, sq]  (+bq per-partition)
                for j in range(SQ // 512):
                    for ec in range(EC):
                        ps = proj_ps.tile([128, 512], f32, tag="ps")
                        for dc in range(DC):
                            nc.tensor.matmul(
                                ps,
                                lhsT=wq_sb[:, dc, ec * 128 : (ec + 1) * 128],
                                rhs=xqT_sb[:, dc, j * 512 : (j + 1) * 512],
                                start=(dc == 0),
                                stop=(dc == DC - 1),
                            )
                        nc.vector.tensor_scalar_add(
                            out=qT_sb[:, ec, j * 512 : (j + 1) * 512],
                            in0=ps,
                            scalar1=bq_sb[:, ec : ec + 1],
                        )

            with (
                tc.tile_pool(name="att", bufs=2) as att,
                tc.tile_pool(name="small", bufs=2) as small,
                tc.tile_pool(name="score_ps", bufs=3, space="PSUM") as score_ps,
                tc.tile_pool(name="tr_ps", bufs=2, space="PSUM") as tr_ps,
                tc.tile_pool(name="attn_ps", bufs=3, space="PSUM") as attn_ps,
            ):
                inv_sqrt_d = float(1.0 / np.sqrt(D))
                for qt in range(QT):
                    qsl = slice(qt * 128, (qt + 1) * 128)
                    P_sb = att.tile([128, S], bf16, tag="P")
                    den4 = small.tile([128, S // 512], f32, tag="den4")
                    for kc in range(S // 512):
                        ps = score_ps.tile([128, 512], f32, tag="score")
                        for ec in range(EC):
                            nc.tensor.matmul(
                                ps,
                                lhsT=qT_sb[:, ec, qsl],
                                rhs=kT_sb[:, ec, kc * 512 : (kc + 1) * 512],
                                start=(ec == 0),
                                stop=(ec == EC - 1),
                            )
                        nc.scalar.activation(
                            out=P_sb[:, kc * 512 : (kc + 1) * 512],
                            in_=ps,
                            func=Act.Exp,
                            scale=inv_sqrt_d,
                            accum_out=den4[:, kc : kc + 1],
                        )
                    recip = small.tile([128, 1], f32, tag="recip")
                    den = small.tile([128, 1], f32, tag="den")
                    nc.vector.reduce_sum(out=den, in_=den4, axis=mybir.AxisListType.X)
                    nc.vector.reciprocal(recip, den)

                    PT_sb = att.tile([128, SC, 128], bf16, tag="PT")
                    for j in range(SC):
                        tp = tr_ps.tile([128, 128], bf16, tag="tr")
                        nc.tensor.transpose(tp, P_sb[:, j * 128 : (j + 1) * 128], ident)
                        nc.vector.tensor_copy(out=PT_sb[:, j, :], in_=tp)

                    xq_sb = att.tile([128, D], f32, tag="xq")
                    nc.sync.dma_start(out=xq_sb, in_=xq_d[qsl, :])
                    ot = att.tile([128, D], f32, tag="ot")
                    for j2 in range(D // 512):
                        pa = attn_ps.tile([128, 512], f32, tag="attn")
                        for j in range(SC):
                            nc.tensor.matmul(
                                pa,
                                lhsT=PT_sb[:, j, :],
                                rhs=v_sb[:, j, j2 * 512 : (j2 + 1) * 512],
                                start=(j == 0),
                                stop=(j == SC - 1),
                            )
                        # out = attn * (1/den) + residual
                        nc.vector.scalar_tensor_tensor(
                            out=ot[:, j2 * 512 : (j2 + 1) * 512],
                            in0=pa,
                            scalar=recip,
                            in1=xq_sb[:, j2 * 512 : (j2 + 1) * 512],
                            op0=Alu.mult,
                            op1=Alu.add,
                        )
                    nc.sync.dma_start(out=out_d[qsl, :], in_=ot)

    nc.compile()
    return nc


def _get_nc():
    if "nc" not in _cache:
        _cache["nc"] = _build()
    return _cache["nc"]


def kernel(embedded, Wq, bq, Wk, bk, Wv, bv):
    import ml_dtypes

    from concourse.bass_utils import run_bass_kernel_spmd

    bf16 = ml_dtypes.bfloat16
    x = np.ascontiguousarray(np.asarray(embedded, dtype=np.float32))
    Wq = np.asarray(Wq, dtype=np.float32)
    Wk = np.asarray(Wk, dtype=np.float32)
    Wv = np.asarray(Wv, dtype=np.float32)
    bq = np.ascontiguousarray(np.asarray(bq, dtype=np.float32))
    bk = np.ascontiguousarray(np.asarray(bk, dtype=np.float32))
    bv = np.ascontiguousarray(np.asarray(bv, dtype=np.float32))

    wqT = np.ascontiguousarray(Wq.T).astype(bf16)
    wkT = np.ascontiguousarray(Wk.T).astype(bf16)
    wvT = np.ascontiguousarray(Wv.T).astype(bf16)
    xT = [np.ascontiguousarray(x[b].T).astype(bf16) for b in range(B)]

    in_maps = []
    for c in range(NCORES):
        b, h = c // 2, c % 2
        qs = slice(h * SQ, (h + 1) * SQ)
        in_maps.append(
            {
                "xT": xT[b],
                "xqT": np.ascontiguousarray(xT[b][:, qs]),
                "xq": np.ascontiguousarray(x[b, qs, :]),
                "wqT": wqT,
                "wkT": wkT,
                "wvT": wvT,
                "bq": bq,
                "bk": bk,
                "bv": bv,
            }
        )

    nc = _get_nc()
    res = run_bass_kernel_spmd(nc, in_maps, core_ids=list(range(NCORES)))
    out = np.empty((B, S, D), dtype=np.float32)
    for c in range(NCORES):
        b, h = c // 2, c % 2
        out[b, h * SQ : (h + 1) * SQ, :] = res.results[c]["out"]
    return out
